# revision 24
# baseline (speedup 1.0000x reference)
"""Trainium2 Bass kernel for nn_BilateralModule (bilateral filter + Canny
NMS + hysteresis), data-parallel across 8 NeuronCores (2 images per core).

Fully unrolled design (no runtime For_i loops except the benchmark rep
loop): the terminal charges ~engine-time + small per-instruction overhead
for straight-line code, while For_i iterations carry a large per-iteration
sync storm (drains + semaphores on all five engines). The bilateral uses
the exact 49-tap circular window with static spatial weights folded into
per-tap immediates, f16 compute in the DVE 4x path (TensorScalarPtr ops),
and splits work across DVE (diff/square/products/accumulate), Act
(|.|, exp) and Pool (channel sums, den accumulate).

Layout: partition p holds image rows 4p..4p+3; dim1 fuses (channel, image)
c*NB+b so engine views stay within 3 free dims.

Also includes a workaround for this walrus build accepting at most ONE
sync-wait per instruction: extra waits are hoisted onto NoOps inserted just
before the instruction on the same engine (identical program-order
semantics).
"""
import numpy as np

import concourse.bass as bass
import concourse.bacc as bacc
import concourse.mybir as mybir
from concourse.mybir import AluOpType as A, ActivationFunctionType as F
from concourse.tile import TileContext

F32 = mybir.dt.float32
F16 = mybir.dt.float16
U8 = mybir.dt.uint8

H = W = 512
PAD = 4
WP = W + 2 * PAD  # 520
WH = W + 2  # 514
MAGIC = 12582912.0
GC = -0.5 / 75.0 ** 2
CS = 32.0  # |diff| pre-scale so cd^2 stays in f16 range
GCS = GC * CS * CS
HIGH_T = 150.0
LOW_T = 50.0
T22 = float(np.tan(np.radians(22.5)))
T67 = float(np.tan(np.radians(67.5)))
HYST_ITERS = 4
NB = 2
NCORES = 8
C6 = 3 * NB

TAPS = [
    (dy, dx)
    for dy in range(-PAD, PAD + 1)
    for dx in range(-PAD, PAD + 1)
    if 0 < dy * dy + dx * dx <= PAD * PAD
]


def build(rep=1):
    nc = bacc.Bacc()
    xp = nc.dram_tensor("xp", [NB, 3, WP, WP], F32, kind="ExternalInput")
    out = nc.dram_tensor("edges", [NB, H, W], F32, kind="ExternalOutput")
    e16d = nc.dram_tensor("e16d", [C6, 8, WP], F16)
    v = nc.vector
    s = nc.scalar
    g = nc.gpsimd

    def stt(eng, out, in0, in1, op0, op1, scalar=0.0):
        eng.scalar_tensor_tensor(out=out, in0=in0, scalar=scalar, in1=in1,
                                 op0=op0, op1=op1)

    # activation float biases / threshold tensors need pre-registered consts
    cvals = [float(GC * r2v) for r2v in sorted({dy * dy + dx * dx for dy, dx in TAPS})]
    cvals += [HIGH_T, LOW_T]
    for i, val in enumerate(cvals):
        t = nc.alloc_sbuf_tensor(f"const-k{i}", [128, 1], F32)
        nc.gpsimd.memset(t.ap(), val)
        nc.const_aps.aps[(F32, val)] = t.ap()

    with TileContext(nc) as tc:
        with tc.For_i(0, rep, 1) as _r:
            pnum_cm = tc.tile_pool(name="pnum", bufs=1)
            pnum = pnum_cm.__enter__()
            num = pnum.tile([128, C6, 4, W], F32, tag="num", name="num")

            pimg_cm = tc.tile_pool(name="pimg", bufs=1)
            pimg = pimg_cm.__enter__()
            imgA = pimg.tile([128, C6, 12, WP], F16, tag="imgA", name="imgA")

            # ---------- load + quantize straight into imgA rows 0:4 ----
            with tc.tile_pool(name="pq", bufs=1) as pq:
                qs = pq.tile([128, NB, 3, 4, WP], F32, tag="qs", name="qs")
                qe = pq.tile([8, NB, 3, 1, WP], F32, tag="qe", name="qe")
                e16 = pq.tile([8, C6, 1, WP], F16, tag="e16", name="e16")
                src = xp[:, :, 0:H, :].rearrange("b c (p r) x -> p b c r x", r=4)
                nc.sync.dma_start(out=qs[:, :, :, :, :], in_=src)
                v.tensor_scalar(qs[:, :, :, :, :], qs[:, :, :, :, :], 0.0, 1.0, A.max, A.min)
                v.tensor_scalar(qs[:, :, :, :, :], qs[:, :, :, :, :], 255.0, MAGIC, A.mult, A.add)
                dst = imgA[:, :, 0:4, :].rearrange("p (b c) r x -> p b c r x", c=3)
                v.tensor_scalar(dst, qs[:, :, :, :, :], MAGIC, None, A.subtract)
                nc.sync.dma_start(
                    out=qe[:, :, :, :, :],
                    in_=xp[:, :, H : H + 8, :].rearrange("b c (p r) x -> p b c r x", r=1),
                )
                v.tensor_scalar(qe[:, :, :, :, :], qe[:, :, :, :, :], 0.0, 1.0, A.max, A.min)
                v.tensor_scalar(qe[:, :, :, :, :], qe[:, :, :, :, :], 255.0, MAGIC, A.mult, A.add)
                e16v = e16[:, :, :, :].rearrange("p (b c) r x -> p b c r x", c=3)
                v.tensor_scalar(e16v, qe[:, :, :, :, :], MAGIC, None, A.subtract)
                nc.sync.dma_start(out=imgA[0:127, :, 4:8, :], in_=imgA[1:128, :, 0:4, :])
                nc.sync.dma_start(out=imgA[0:126, :, 8:12, :], in_=imgA[2:128, :, 0:4, :])
                # corner rows via DRAM staging: one store + three reordered loads
                nc.sync.dma_start(out=e16d[:, :, :].rearrange("c r x -> r c x"),
                                  in_=e16[:, :, 0, :])
                nc.sync.dma_start(out=imgA[127:128, :, 4:8, :], in_=e16d[:, 0:4, :])
                nc.sync.dma_start(out=imgA[126:127, :, 8:12, :], in_=e16d[:, 0:4, :])
                nc.sync.dma_start(out=imgA[127:128, :, 8:12, :], in_=e16d[:, 4:8, :])

            # ---------- bilateral: 48 unrolled taps + center ----------
            # f16 only where values are exact integers (image, |diff|, channel
            # sums <= 765); everything from Square onward is f32.
            ctr6 = imgA[:, :, 4:8, 4 : 4 + W]
            v.tensor_tensor(out=num[:, :, :, :], in0=ctr6, in1=ctr6, op=A.max)
            pt_cm = tc.tile_pool(name="pt", bufs=1)
            pt = pt_cm.__enter__()
            if True:
                den = pt.tile([128, NB, 4, W], F32, tag="den", name="den")
                g.memset(den[:, :, :, :], 1.0)
                dt_ = pt.tile([128, C6, 4, W], F16, tag="dt", name="dt")
                cds = pt.tile([128, NB, 4, W], F16, tag="cds", name="cds")
                sqw = pt.tile([128, NB, 4, W], F32, tag="sqw", name="sqw")
                pr = pt.tile([128, NB, 4, W], F32, tag="pr", name="pr")
                for dy, dx in TAPS:
                    sy, sx = dy + PAD, dx + PAD
                    bias_r2 = float(GC * (dy * dy + dx * dx))
                    sh6 = imgA[:, :, sy : sy + 4, sx : sx + W]
                    v.tensor_tensor(out=dt_[:, :, :, :], in0=sh6, in1=ctr6, op=A.subtract)
                    s.activation(out=dt_[:, :, :, :], in_=dt_[:, :, :, :], func=F.Abs)
                    g.tensor_tensor(out=cds[:, :, :, :], in0=dt_[:, 0::3, :, :],
                                    in1=dt_[:, 1::3, :, :], op=A.add)
                    g.tensor_tensor(out=cds[:, :, :, :], in0=cds[:, :, :, :],
                                    in1=dt_[:, 2::3, :, :], op=A.add)
                    s.activation(out=sqw[:, :, :, :], in_=cds[:, :, :, :], func=F.Square)
                    # wt = exp(GC*cd^2 + GC*r2): full bilateral weight
                    s.activation(out=sqw[:, :, :, :], in_=sqw[:, :, :, :], func=F.Exp,
                                 scale=GC, bias=bias_r2)
                    for c in range(3):
                        sh_c = imgA[:, c::3, sy : sy + 4, sx : sx + W]
                        v.tensor_tensor(out=pr[:, :, :, :], in0=sqw[:, :, :, :],
                                        in1=sh_c, op=A.mult)
                        v.tensor_tensor(out=num[:, c::3, :, :], in0=num[:, c::3, :, :],
                                        in1=pr[:, :, :, :], op=A.add)
                    g.tensor_tensor(out=den[:, :, :, :], in0=sqw[:, :, :, :],
                                    in1=den[:, :, :, :], op=A.add)
            # ---------- divide in place: num becomes filt (f32) ----------
            rcp = pr  # pr is dead after the last tap
            v.reciprocal(out=rcp[:, :, :, :], in_=den[:, :, :, :])
            for c in range(3):
                v.tensor_tensor(out=num[:, c::3, :, :], in0=num[:, c::3, :, :],
                                in1=rcp[:, :, :, :], op=A.mult)
            pt_cm.__exit__(None, None, None)
            pimg_cm.__exit__(None, None, None)  # free imgA

            # ---------- NMS: channel-sequential, both images batched ------
            with tc.tile_pool(name="psel", bufs=1, side="right") as psel:
                mags = psel.tile([128, NB, 4, W], F32, tag="mags", name="mags")
                nmst = psel.tile([128, NB, 4, W], F32, tag="nmst", name="nmst")
                d0m = psel.tile([128, NB, 4, W], U8, tag="d0m", name="d0m")
                d45 = psel.tile([128, NB, 4, W], U8, tag="d45", name="d45")
                d90 = psel.tile([128, NB, 4, W], U8, tag="d90", name="d90")
                pgxy_cm = tc.tile_pool(name="pgxy", bufs=1, side="right")
                pgxy = pgxy_cm.__enter__()
                gxs = pgxy.tile([128, NB, 4, W], F32, tag="gxs", name="gxs")
                gys = pgxy.tile([128, NB, 4, W], F32, tag="gys", name="gys")
                with tc.tile_pool(name="pch", bufs=1, side="right") as pch:
                    tN = pch.tile([128, NB, 1, W], F32, tag="tN", name="tN")
                    bN = pch.tile([128, NB, 1, W], F32, tag="bN", name="bN")
                    sm = pch.tile([128, NB, 6, W], F32, tag="sm", name="sm")
                    gx = pch.tile([128, NB, 4, W], F32, tag="gx", name="gx")
                    gy = pch.tile([128, NB, 4, W], F32, tag="gy", name="gy")
                    mg = pch.tile([128, NB, 4, W], F32, tag="mg", name="mg")
                    sel = d0m  # scratch reuse: d0m is only written in the dir phase

                    def tt(out_, a, b_, op=A.add):
                        v.tensor_tensor(out=out_, in0=a, in1=b_, op=op)

                    for c in range(3):
                        f = num[:, c::3, :, :]
                        nc.sync.dma_start(out=tN[1:128, :, 0, :], in_=f[0:127, :, 3, :])
                        v.tensor_copy(out=tN[0:1, :, 0, :], in_=f[0:1, :, 0, :])
                        nc.sync.dma_start(out=bN[0:127, :, 0, :], in_=f[1:128, :, 0, :])
                        nc.sync.dma_start(out=bN[127:128, :, 0, :], in_=f[127:128, :, 3, :])
                        # vertical 1-2-1 -> sm rows 0:4 (syt): 2f + up + down
                        tt(sm[:, :, 1:3, :], f[:, :, 1:3, :], f[:, :, 1:3, :])
                        tt(sm[:, :, 1:3, :], sm[:, :, 1:3, :], f[:, :, 0:2, :])
                        tt(sm[:, :, 1:3, :], sm[:, :, 1:3, :], f[:, :, 2:4, :])
                        tt(sm[:, :, 0:1, :], f[:, :, 0:1, :], f[:, :, 0:1, :])
                        tt(sm[:, :, 0:1, :], sm[:, :, 0:1, :], tN[:, :, 0:1, :])
                        tt(sm[:, :, 0:1, :], sm[:, :, 0:1, :], f[:, :, 1:2, :])
                        tt(sm[:, :, 3:4, :], f[:, :, 3:4, :], f[:, :, 3:4, :])
                        tt(sm[:, :, 3:4, :], sm[:, :, 3:4, :], f[:, :, 2:3, :])
                        tt(sm[:, :, 3:4, :], sm[:, :, 3:4, :], bN[:, :, 0:1, :])
                        syt = sm[:, :, 0:4, :]
                        tt(gx[:, :, :, 1 : W - 1], syt[:, :, :, 2:W], syt[:, :, :, 0 : W - 2], A.subtract)
                        tt(gx[:, :, :, 0:1], syt[:, :, :, 1:2], syt[:, :, :, 0:1], A.subtract)
                        tt(gx[:, :, :, W - 1 : W], syt[:, :, :, W - 1 : W], syt[:, :, :, W - 2 : W - 1], A.subtract)
                        # horizontal 1-2-1 over 6 virtual rows -> sm (sxh);
                        # syt rows consumed above before being overwritten
                        tt(sm[:, :, 1:5, 1 : W - 1], f[:, :, :, 1 : W - 1], f[:, :, :, 1 : W - 1])
                        tt(sm[:, :, 1:5, 1 : W - 1], sm[:, :, 1:5, 1 : W - 1], f[:, :, :, 0 : W - 2])
                        tt(sm[:, :, 1:5, 1 : W - 1], sm[:, :, 1:5, 1 : W - 1], f[:, :, :, 2:W])
                        tt(sm[:, :, 1:5, 0:1], f[:, :, :, 0:1], f[:, :, :, 0:1])
                        tt(sm[:, :, 1:5, 0:1], sm[:, :, 1:5, 0:1], f[:, :, :, 0:1])
                        tt(sm[:, :, 1:5, 0:1], sm[:, :, 1:5, 0:1], f[:, :, :, 1:2])
                        tt(sm[:, :, 1:5, W - 1 : W], f[:, :, :, W - 1 : W], f[:, :, :, W - 1 : W])
                        tt(sm[:, :, 1:5, W - 1 : W], sm[:, :, 1:5, W - 1 : W], f[:, :, :, W - 1 : W])
                        tt(sm[:, :, 1:5, W - 1 : W], sm[:, :, 1:5, W - 1 : W], f[:, :, :, W - 2 : W - 1])
                        for rowdst, rowsrc in ((sm[:, :, 0:1, :], tN[:, :, 0:1, :]),
                                               (sm[:, :, 5:6, :], bN[:, :, 0:1, :])):
                            stt(v, rowdst[:, :, :, 1 : W - 1], rowsrc[:, :, :, 1 : W - 1],
                                rowsrc[:, :, :, 0 : W - 2], A.mult, A.add, 2.0)
                            stt(v, rowdst[:, :, :, 1 : W - 1], rowdst[:, :, :, 1 : W - 1],
                                rowsrc[:, :, :, 2:W], A.add, A.add)
                            stt(v, rowdst[:, :, :, 0:1], rowsrc[:, :, :, 0:1],
                                rowsrc[:, :, :, 1:2], A.mult, A.add, 3.0)
                            stt(v, rowdst[:, :, :, W - 1 : W], rowsrc[:, :, :, W - 1 : W],
                                rowsrc[:, :, :, W - 2 : W - 1], A.mult, A.add, 3.0)
                        tt(gy[:, :, :, :], sm[:, :, 2:6, :], sm[:, :, 0:4, :], A.subtract)
                        ayb = sm[:, :, 0:4, :]  # scratch: sm fully consumed by gy
                        s.activation(out=mg[:, :, :, :], in_=gx[:, :, :, :], func=F.Abs)
                        s.activation(out=ayb, in_=gy[:, :, :, :], func=F.Abs)
                        tt(mg[:, :, :, :], mg[:, :, :, :], ayb)
                        if c == 0:
                            v.tensor_copy(out=gxs[:, :, :, :], in_=gx[:, :, :, :])
                            v.tensor_copy(out=gys[:, :, :, :], in_=gy[:, :, :, :])
                            v.tensor_copy(out=mags[:, :, :, :], in_=mg[:, :, :, :])
                        else:
                            stt(v, sel[:, :, :, :], mg[:, :, :, :], mags[:, :, :, :],
                                A.bypass, A.is_gt)
                            v.copy_predicated(out=gxs[:, :, :, :], mask=sel[:, :, :, :], data=gx[:, :, :, :])
                            v.copy_predicated(out=gys[:, :, :, :], mask=sel[:, :, :, :], data=gy[:, :, :, :])
                            v.copy_predicated(out=mags[:, :, :, :], mask=sel[:, :, :, :], data=mg[:, :, :, :])
                pnum_cm.__exit__(None, None, None)  # free num

                # direction masks
                with tc.tile_pool(name="pdir", bufs=1, side="right") as pdir:
                    u = pdir.tile([128, NB, 4, W], F32, tag="u", name="u")
                    ax = pdir.tile([128, NB, 4, W], F32, tag="ax", name="ax")
                    tA = pdir.tile([128, NB, 4, W], F32, tag="tA", name="tA")
                    TA = pdir.tile([128, NB, 4, W], F32, tag="TA", name="TA")
                    t1 = pdir.tile([128, NB, 4, W], F32, tag="t1", name="t1")
                    b1 = pdir.tile([128, NB, 4, W], U8, tag="b1", name="b1")
                    b2 = pdir.tile([128, NB, 4, W], U8, tag="b2", name="b2")
                    v.tensor_scalar(ax[:, :, :, :], gxs[:, :, :, :], 0.0, 2.0, A.is_ge, A.mult)
                    stt(v, u[:, :, :, :], ax[:, :, :, :], gys[:, :, :, :], A.bypass, A.mult)
                    stt(v, u[:, :, :, :], u[:, :, :, :], gys[:, :, :, :], A.bypass, A.subtract)
                    s.activation(out=ax[:, :, :, :], in_=gxs[:, :, :, :], func=F.Abs)
                    v.tensor_scalar(tA[:, :, :, :], ax[:, :, :, :], T22, None, A.mult)
                    v.tensor_scalar(TA[:, :, :, :], ax[:, :, :, :], T67, None, A.mult)
                    # d0: -tA <= u < tA   (or mag == 0)
                    stt(v, t1[:, :, :, :], u[:, :, :, :], tA[:, :, :, :], A.bypass, A.add)
                    v.tensor_scalar(b1[:, :, :, :], t1[:, :, :, :], 0.0, None, A.is_ge)
                    stt(v, t1[:, :, :, :], u[:, :, :, :], tA[:, :, :, :], A.bypass, A.subtract)
                    v.tensor_scalar(b2[:, :, :, :], t1[:, :, :, :], 0.0, None, A.is_lt)
                    stt(v, d0m[:, :, :, :], b1[:, :, :, :], b2[:, :, :, :], A.bypass, A.logical_and)
                    v.tensor_scalar(b1[:, :, :, :], mags[:, :, :, :], 0.0, None, A.is_equal)
                    stt(v, d0m[:, :, :, :], d0m[:, :, :, :], b1[:, :, :, :], A.bypass, A.logical_or)
                    # d45: tA <= u < TA   (t1 still holds u - tA)
                    v.tensor_scalar(b1[:, :, :, :], t1[:, :, :, :], 0.0, None, A.is_ge)
                    stt(v, t1[:, :, :, :], u[:, :, :, :], TA[:, :, :, :], A.bypass, A.subtract)
                    v.tensor_scalar(b2[:, :, :, :], t1[:, :, :, :], 0.0, None, A.is_lt)
                    stt(v, d45[:, :, :, :], b1[:, :, :, :], b2[:, :, :, :], A.bypass, A.logical_and)
                    # d90: u >= TA or u < -TA   (t1 still holds u - TA)
                    v.tensor_scalar(b1[:, :, :, :], t1[:, :, :, :], 0.0, None, A.is_ge)
                    stt(v, t1[:, :, :, :], u[:, :, :, :], TA[:, :, :, :], A.bypass, A.add)
                    v.tensor_scalar(b2[:, :, :, :], t1[:, :, :, :], 0.0, None, A.is_lt)
                    stt(v, d90[:, :, :, :], b1[:, :, :, :], b2[:, :, :, :], A.bypass, A.logical_or)
                pgxy_cm.__exit__(None, None, None)  # free gxs/gys

                # neighbor picks + suppression (per image: copy_predicated
                # and its neighbor views must stay within 2 free dims)
                with tc.tile_pool(name="pnbr", bufs=1, side="right") as pnbr:
                    mh = pnbr.tile([128, 6, WH], F32, tag="mh", name="mh")
                    n1 = pnbr.tile([128, 4, W], F32, tag="n1", name="n1")
                    n2 = pnbr.tile([128, 4, W], F32, tag="n2", name="n2")
                    for b in range(NB):
                        mb = mags[:, b, :, :]
                        v.memset(mh[:, :, :], 0.0)
                        v.tensor_copy(out=mh[:, 1:5, 1 : 1 + W], in_=mb)
                        nc.sync.dma_start(out=mh[1:128, 0:1, 1 : 1 + W], in_=mb[0:127, 3:4, :])
                        nc.sync.dma_start(out=mh[0:127, 5:6, 1 : 1 + W], in_=mb[1:128, 0:1, :])

                        def nbr(dy2, dx2):
                            return mh[:, 1 + dy2 : 5 + dy2, 1 + dx2 : 1 + dx2 + W]

                        v.tensor_copy(out=n1[:, :, :], in_=nbr(-1, -1))
                        v.copy_predicated(out=n1[:, :, :], mask=d90[:, b, :, :], data=nbr(-1, 0))
                        v.copy_predicated(out=n1[:, :, :], mask=d45[:, b, :, :], data=nbr(-1, 1))
                        v.copy_predicated(out=n1[:, :, :], mask=d0m[:, b, :, :], data=nbr(0, 1))
                        v.tensor_copy(out=n2[:, :, :], in_=nbr(1, 1))
                        v.copy_predicated(out=n2[:, :, :], mask=d90[:, b, :, :], data=nbr(1, 0))
                        v.copy_predicated(out=n2[:, :, :], mask=d45[:, b, :, :], data=nbr(1, -1))
                        v.copy_predicated(out=n2[:, :, :], mask=d0m[:, b, :, :], data=nbr(0, -1))
                        stt(v, n1[:, :, :], mb, n1[:, :, :], A.bypass, A.is_ge)
                        stt(v, n2[:, :, :], mb, n2[:, :, :], A.bypass, A.is_ge)
                        stt(v, n1[:, :, :], n1[:, :, :], n2[:, :, :], A.bypass, A.mult)
                        stt(v, nmst[:, b, :, :], mb, n1[:, :, :], A.bypass, A.mult)

            # ---------- hysteresis (both images batched) ----------
            with tc.tile_pool(name="phy", bufs=1) as phy:
                st = phy.tile([128, NB, 4, WH], F16, tag="st", name="st")
                sc = phy.tile([128, NB, 4, WH], F16, tag="sc", name="sc")
                wk = phy.tile([128, NB, 4, WH], F16, tag="wk", name="wk")
                hdil = phy.tile([128, NB, 4, WH], F16, tag="hdil", name="hdil")
                vdil = phy.tile([128, NB, 6, WH], F16, tag="vdil", name="vdil")
                dil = phy.tile([128, NB, 4, WH], F16, tag="dil", name="dil")
                v.memset(st[:, :, :, :], 0.0)
                v.memset(wk[:, :, :, :], 0.0)
                v.memset(hdil[:, :, :, :], 0.0)
                v.memset(vdil[:, :, :, :], 0.0)
                hbc = nc.const_aps.tensor(HIGH_T, (128, NB, 4, W))
                lbc = nc.const_aps.tensor(LOW_T, (128, NB, 4, W))
                v.tensor_tensor(out=st[:, :, :, 1 : 1 + W], in0=nmst[:, :, :, :],
                                in1=hbc, op=A.is_gt)
                v.tensor_tensor(out=wk[:, :, :, 1 : 1 + W], in0=nmst[:, :, :, :],
                                in1=lbc, op=A.is_gt)
                st2d = st[:, :, :, :].rearrange("p i a x -> p (i a x)")
                sc2d = sc[:, :, :, :].rearrange("p i a x -> p (i a x)")
                wk2d = wk[:, :, :, :].rearrange("p i a x -> p (i a x)")
                for _it in range(HYST_ITERS):
                    v.tensor_tensor_scan(out=sc2d, data0=wk2d, data1=st2d,
                                         initial=0.0, op0=A.mult, op1=A.max)
                    v.tensor_tensor_scan(out=st2d[:, ::-1], data0=wk2d[:, ::-1],
                                         data1=sc2d[:, ::-1], initial=0.0, op0=A.mult, op1=A.max)
                    v.tensor_tensor(out=hdil[:, :, :, 1 : 1 + W], in0=st[:, :, :, 0:W],
                                    in1=st[:, :, :, 2 : 2 + W], op=A.max)
                    v.tensor_tensor(out=vdil[:, :, 1:5, 1 : 1 + W], in0=hdil[:, :, :, 1 : 1 + W],
                                    in1=st[:, :, :, 1 : 1 + W], op=A.max)
                    if _it < HYST_ITERS - 1:
                        nc.sync.dma_start(out=vdil[1:128, :, 0:1, 1 : 1 + W], in_=vdil[0:127, :, 4:5, 1 : 1 + W])
                        nc.sync.dma_start(out=vdil[0:127, :, 5:6, 1 : 1 + W], in_=vdil[1:128, :, 1:2, 1 : 1 + W])
                    stt(v, dil[:, :, :, :], vdil[:, :, 0:4, :], vdil[:, :, 2:6, :], A.bypass, A.max)
                    stt(v, dil[:, :, :, :], dil[:, :, :, :], vdil[:, :, 1:5, :], A.bypass, A.max)
                    stt(v, dil[:, :, :, :], dil[:, :, :, :], wk[:, :, :, :], A.bypass, A.mult)
                    stt(v, st[:, :, :, :], st[:, :, :, :], dil[:, :, :, :], A.bypass, A.max)
                    # (all operands above are contiguous or row-range views that
                    # canonically merge to <=2 free dims)
                o32 = phy.tile([128, NB, 4, W], F32, tag="o32", name="o32")
                v.tensor_tensor(out=o32[:, :, :, :], in0=st[:, :, :, 1 : 1 + W],
                                in1=st[:, :, :, 1 : 1 + W], op=A.max)
                nc.sync.dma_start(
                    out=out[:, :, :].rearrange("b (p r) x -> p b r x", r=4),
                    in_=o32[:, :, :, :],
                )
    nc.finalize()
    return nc


# ---------------------------------------------------------------------------
# walrus 1-sync-wait-per-instruction workaround (BIR JSON post-pass)
# ---------------------------------------------------------------------------
import json as _json

_ws_counter = [0]


def _split_instruction_list(instrs):
    out = []
    for ins in instrs:
        si = ins.get("sync_info")
        waits = (si or {}).get("on_wait") or []
        if len(waits) > 1:
            for wcond in waits[:-1]:
                _ws_counter[0] += 1
                out.append({
                    "debug": ins.get("debug", 0),
                    "engine": ins["engine"],
                    "ins": [],
                    "name": f"I-waitsplit-{_ws_counter[0]}",
                    "opcode": "NoOp",
                    "outs": [],
                    "sync_info": {"on_wait": [wcond], "on_update": []},
                })
            si = dict(si)
            si["on_wait"] = [waits[-1]]
            ins = dict(ins)
            ins["sync_info"] = si
        out.append(ins)
    return out


def _walk_split(obj):
    if isinstance(obj, dict):
        for k, val in obj.items():
            if k == "instructions" and isinstance(val, list):
                obj[k] = _split_instruction_list(val)
            else:
                _walk_split(val)
    elif isinstance(obj, list):
        for val in obj:
            _walk_split(val)


def _split_multiwait_bir(bir_json):
    j = _json.loads(bir_json)
    _walk_split(j)
    return _json.dumps(j).encode()


_patched = [False]


def _install_bir_patch():
    if _patched[0]:
        return
    _patched[0] = True
    import concourse.bass_utils as bu

    orig = bu.compile_bir_kernel

    def patched(bir_json, tmpdir, neff_name="file.neff"):
        return orig(_split_multiwait_bir(bir_json), tmpdir, neff_name)

    bu.compile_bir_kernel = patched
    try:
        import concourse.bass2jax as b2j

        b2j.compile_bir_kernel = patched
    except Exception:
        pass


# ---------------------------------------------------------------------------
# host entry point
# ---------------------------------------------------------------------------
_cache = {}


def _get_program(rep=1):
    key = ("nc", rep)
    if key not in _cache:
        _install_bir_patch()
        _cache[key] = build(rep=rep)
    return _cache[key]


def make_in_maps(x):
    x = np.asarray(x, dtype=np.float32)
    xpad = np.pad(x, ((0, 0), (0, 0), (PAD, PAD), (PAD, PAD)), mode="reflect")
    return [
        {"xp": np.ascontiguousarray(xpad[i * NB : (i + 1) * NB])}
        for i in range(NCORES)
    ]


def kernel(x):
    """x: [16,3,512,512] float32 -> edges [16,1,512,512] float32."""
    from concourse.bass_utils import run_bass_kernel_spmd

    x = np.asarray(x, dtype=np.float32)
    B = x.shape[0]
    assert x.shape == (NCORES * NB, 3, H, W), x.shape
    nc = _get_program()
    in_maps = make_in_maps(x)
    res = run_bass_kernel_spmd(nc, in_maps, core_ids=list(range(NCORES)))
    out = np.empty((B, 1, H, W), np.float32)
    for i in range(NCORES):
        out[i * NB : (i + 1) * NB, 0] = res.results[i]["edges"]
    return out


# revision 26
# speedup vs baseline: 4.0787x; 4.0787x over previous
"""Trainium2 Bass kernel for nn_BilateralModule (bilateral filter + Canny
NMS + hysteresis), data-parallel across 8 NeuronCores (2 images per core).

Fully unrolled design (no runtime For_i loops except the benchmark rep
loop): the terminal charges ~engine-time + small per-instruction overhead
for straight-line code, while For_i iterations carry a large per-iteration
sync storm (drains + semaphores on all five engines). The bilateral uses
the exact 49-tap circular window with static spatial weights folded into
per-tap immediates, f16 compute in the DVE 4x path (TensorScalarPtr ops),
and splits work across DVE (diff/square/products/accumulate), Act
(|.|, exp) and Pool (channel sums, den accumulate).

Layout: partition p holds image rows 4p..4p+3; dim1 fuses (channel, image)
c*NB+b so engine views stay within 3 free dims.

Also includes a workaround for this walrus build accepting at most ONE
sync-wait per instruction: extra waits are hoisted onto NoOps inserted just
before the instruction on the same engine (identical program-order
semantics).
"""
import numpy as np

import concourse.bass as bass
import concourse.bacc as bacc
import concourse.mybir as mybir
from concourse.mybir import AluOpType as A, ActivationFunctionType as F
from concourse.tile import TileContext

F32 = mybir.dt.float32
F16 = mybir.dt.float16
U8 = mybir.dt.uint8

H = W = 512
PAD = 4
WP = W + 2 * PAD  # 520
WH = W + 2  # 514
MAGIC = 12582912.0
GC = -0.5 / 75.0 ** 2
CS = 32.0  # |diff| pre-scale so cd^2 stays in f16 range
GCS = GC * CS * CS
HIGH_T = 150.0
LOW_T = 50.0
T22 = float(np.tan(np.radians(22.5)))
T67 = float(np.tan(np.radians(67.5)))
HYST_ITERS = 4
HYST_INIT_TT = True     # init st/wk via const-broadcast TT (False: ts+DMA)
HYST_SKIP_LAST_X = True  # skip cross-partition exchange on last iter
NB = 2
NCORES = 8
C6 = 3 * NB

TAPS = [
    (dy, dx)
    for dy in range(-PAD, PAD + 1)
    for dx in range(-PAD, PAD + 1)
    if 0 < dy * dy + dx * dx <= PAD * PAD
]


def build(rep=1):
    nc = bacc.Bacc()
    xp = nc.dram_tensor("xp", [NB, 3, WP, WP], F32, kind="ExternalInput")
    out = nc.dram_tensor("edges", [NB, H, W], F32, kind="ExternalOutput")
    e16d = nc.dram_tensor("e16d", [C6, 8, WP], F16)
    v = nc.vector
    s = nc.scalar
    g = nc.gpsimd

    def stt(eng, out, in0, in1, op0, op1, scalar=0.0):
        eng.scalar_tensor_tensor(out=out, in0=in0, scalar=scalar, in1=in1,
                                 op0=op0, op1=op1)

    # activation float biases / threshold tensors need pre-registered consts
    cvals = [float(GC * r2v) for r2v in sorted({dy * dy + dx * dx for dy, dx in TAPS})]
    cvals += [HIGH_T, LOW_T]
    for i, val in enumerate(cvals):
        t = nc.alloc_sbuf_tensor(f"const-k{i}", [128, 1], F32)
        nc.gpsimd.memset(t.ap(), val)
        nc.const_aps.aps[(F32, val)] = t.ap()

    with TileContext(nc) as tc:
        with tc.For_i(0, rep, 1) as _r:
            pnum_cm = tc.tile_pool(name="pnum", bufs=1)
            pnum = pnum_cm.__enter__()
            num = pnum.tile([128, C6, 4, W], F32, tag="num", name="num")

            pimg_cm = tc.tile_pool(name="pimg", bufs=1)
            pimg = pimg_cm.__enter__()
            imgA = pimg.tile([128, C6, 12, WP], F16, tag="imgA", name="imgA")

            # ---------- load + quantize straight into imgA rows 0:4 ----
            with tc.tile_pool(name="pq", bufs=1) as pq:
                qs = pq.tile([128, NB, 3, 4, WP], F32, tag="qs", name="qs")
                qe = pq.tile([8, NB, 3, 1, WP], F32, tag="qe", name="qe")
                e16 = pq.tile([8, C6, 1, WP], F16, tag="e16", name="e16")
                src = xp[:, :, 0:H, :].rearrange("b c (p r) x -> p b c r x", r=4)
                nc.sync.dma_start(out=qs[:, :, :, :, :], in_=src)
                v.tensor_scalar(qs[:, :, :, :, :], qs[:, :, :, :, :], 0.0, 1.0, A.max, A.min)
                v.tensor_scalar(qs[:, :, :, :, :], qs[:, :, :, :, :], 255.0, MAGIC, A.mult, A.add)
                dst = imgA[:, :, 0:4, :].rearrange("p (b c) r x -> p b c r x", c=3)
                v.tensor_scalar(dst, qs[:, :, :, :, :], MAGIC, None, A.subtract)
                nc.sync.dma_start(
                    out=qe[:, :, :, :, :],
                    in_=xp[:, :, H : H + 8, :].rearrange("b c (p r) x -> p b c r x", r=1),
                )
                v.tensor_scalar(qe[:, :, :, :, :], qe[:, :, :, :, :], 0.0, 1.0, A.max, A.min)
                v.tensor_scalar(qe[:, :, :, :, :], qe[:, :, :, :, :], 255.0, MAGIC, A.mult, A.add)
                e16v = e16[:, :, :, :].rearrange("p (b c) r x -> p b c r x", c=3)
                v.tensor_scalar(e16v, qe[:, :, :, :, :], MAGIC, None, A.subtract)
                nc.sync.dma_start(out=imgA[0:127, :, 4:8, :], in_=imgA[1:128, :, 0:4, :])
                nc.sync.dma_start(out=imgA[0:126, :, 8:12, :], in_=imgA[2:128, :, 0:4, :])
                # corner rows via DRAM staging: one store + three reordered loads
                nc.sync.dma_start(out=e16d[:, :, :].rearrange("c r x -> r c x"),
                                  in_=e16[:, :, 0, :])
                nc.sync.dma_start(out=imgA[127:128, :, 4:8, :], in_=e16d[:, 0:4, :])
                nc.sync.dma_start(out=imgA[126:127, :, 8:12, :], in_=e16d[:, 0:4, :])
                nc.sync.dma_start(out=imgA[127:128, :, 8:12, :], in_=e16d[:, 4:8, :])

            # ---------- bilateral: 48 unrolled taps + center ----------
            # f16 only where values are exact integers (image, |diff|, channel
            # sums <= 765); everything from Square onward is f32.
            ctr6 = imgA[:, :, 4:8, 4 : 4 + W]
            v.tensor_tensor(out=num[:, :, :, :], in0=ctr6, in1=ctr6, op=A.max)
            pt_cm = tc.tile_pool(name="pt", bufs=1)
            pt = pt_cm.__enter__()
            if True:
                den = pt.tile([128, NB, 4, W], F32, tag="den", name="den")
                g.memset(den[:, :, :, :], 1.0)
                dt_ = pt.tile([128, C6, 4, W], F16, tag="dt", name="dt")
                cds = pt.tile([128, NB, 4, W], F16, tag="cds", name="cds")
                sqw = pt.tile([128, NB, 4, W], F32, tag="sqw", name="sqw")
                pr = pt.tile([128, NB, 4, W], F32, tag="pr", name="pr")
                for dy, dx in TAPS:
                    sy, sx = dy + PAD, dx + PAD
                    bias_r2 = float(GC * (dy * dy + dx * dx))
                    sh6 = imgA[:, :, sy : sy + 4, sx : sx + W]
                    v.tensor_tensor(out=dt_[:, :, :, :], in0=sh6, in1=ctr6, op=A.subtract)
                    s.activation(out=dt_[:, :, :, :], in_=dt_[:, :, :, :], func=F.Abs)
                    g.tensor_tensor(out=cds[:, :, :, :], in0=dt_[:, 0::3, :, :],
                                    in1=dt_[:, 1::3, :, :], op=A.add)
                    g.tensor_tensor(out=cds[:, :, :, :], in0=cds[:, :, :, :],
                                    in1=dt_[:, 2::3, :, :], op=A.add)
                    s.activation(out=sqw[:, :, :, :], in_=cds[:, :, :, :], func=F.Square)
                    # wt = exp(GC*cd^2 + GC*r2): full bilateral weight
                    s.activation(out=sqw[:, :, :, :], in_=sqw[:, :, :, :], func=F.Exp,
                                 scale=GC, bias=bias_r2)
                    for c in range(3):
                        sh_c = imgA[:, c::3, sy : sy + 4, sx : sx + W]
                        v.tensor_tensor(out=pr[:, :, :, :], in0=sqw[:, :, :, :],
                                        in1=sh_c, op=A.mult)
                        v.tensor_tensor(out=num[:, c::3, :, :], in0=num[:, c::3, :, :],
                                        in1=pr[:, :, :, :], op=A.add)
                    g.tensor_tensor(out=den[:, :, :, :], in0=sqw[:, :, :, :],
                                    in1=den[:, :, :, :], op=A.add)
            # ---------- divide in place: num becomes filt (f32) ----------
            rcp = pr  # pr is dead after the last tap
            v.reciprocal(out=rcp[:, :, :, :], in_=den[:, :, :, :])
            for c in range(3):
                v.tensor_tensor(out=num[:, c::3, :, :], in0=num[:, c::3, :, :],
                                in1=rcp[:, :, :, :], op=A.mult)
            pt_cm.__exit__(None, None, None)
            pimg_cm.__exit__(None, None, None)  # free imgA

            # ---------- NMS: channel-sequential, both images batched ------
            with tc.tile_pool(name="psel", bufs=1, side="right") as psel:
                mags = psel.tile([128, NB, 4, W], F32, tag="mags", name="mags")
                nmst = psel.tile([128, NB, 4, W], F32, tag="nmst", name="nmst")
                d0m = psel.tile([128, NB, 4, W], U8, tag="d0m", name="d0m")
                d45 = psel.tile([128, NB, 4, W], U8, tag="d45", name="d45")
                d90 = psel.tile([128, NB, 4, W], U8, tag="d90", name="d90")
                pgxy_cm = tc.tile_pool(name="pgxy", bufs=1, side="right")
                pgxy = pgxy_cm.__enter__()
                gxs = pgxy.tile([128, NB, 4, W], F32, tag="gxs", name="gxs")
                gys = pgxy.tile([128, NB, 4, W], F32, tag="gys", name="gys")
                with tc.tile_pool(name="pch", bufs=1, side="right") as pch:
                    tN = pch.tile([128, NB, 1, W], F32, tag="tN", name="tN")
                    bN = pch.tile([128, NB, 1, W], F32, tag="bN", name="bN")
                    sm = pch.tile([128, NB, 6, W], F32, tag="sm", name="sm")
                    gx = pch.tile([128, NB, 4, W], F32, tag="gx", name="gx")
                    gy = pch.tile([128, NB, 4, W], F32, tag="gy", name="gy")
                    mg = pch.tile([128, NB, 4, W], F32, tag="mg", name="mg")
                    sel = d0m  # scratch reuse: d0m is only written in the dir phase

                    def tt(out_, a, b_, op=A.add):
                        v.tensor_tensor(out=out_, in0=a, in1=b_, op=op)

                    for c in range(3):
                        f = num[:, c::3, :, :]
                        v.tensor_copy(out=tN[:, :, 0, :], in_=f[:, :, 0, :])
                        v.tensor_copy(out=bN[:, :, 0, :], in_=f[:, :, 3, :])
                        nc.sync.dma_start(out=tN[1:128, :, 0, :], in_=f[0:127, :, 3, :])
                        nc.sync.dma_start(out=bN[0:127, :, 0, :], in_=f[1:128, :, 0, :])
                        # vertical 1-2-1 -> sm rows 0:4 (syt): 2f + up + down
                        tt(sm[:, :, 1:3, :], f[:, :, 1:3, :], f[:, :, 1:3, :])
                        tt(sm[:, :, 1:3, :], sm[:, :, 1:3, :], f[:, :, 0:2, :])
                        tt(sm[:, :, 1:3, :], sm[:, :, 1:3, :], f[:, :, 2:4, :])
                        tt(sm[:, :, 0:1, :], f[:, :, 0:1, :], f[:, :, 0:1, :])
                        tt(sm[:, :, 0:1, :], sm[:, :, 0:1, :], tN[:, :, 0:1, :])
                        tt(sm[:, :, 0:1, :], sm[:, :, 0:1, :], f[:, :, 1:2, :])
                        tt(sm[:, :, 3:4, :], f[:, :, 3:4, :], f[:, :, 3:4, :])
                        tt(sm[:, :, 3:4, :], sm[:, :, 3:4, :], f[:, :, 2:3, :])
                        tt(sm[:, :, 3:4, :], sm[:, :, 3:4, :], bN[:, :, 0:1, :])
                        syt = sm[:, :, 0:4, :]
                        tt(gx[:, :, :, 1 : W - 1], syt[:, :, :, 2:W], syt[:, :, :, 0 : W - 2], A.subtract)
                        tt(gx[:, :, :, 0:1], syt[:, :, :, 1:2], syt[:, :, :, 0:1], A.subtract)
                        tt(gx[:, :, :, W - 1 : W], syt[:, :, :, W - 1 : W], syt[:, :, :, W - 2 : W - 1], A.subtract)
                        # horizontal 1-2-1 over 6 virtual rows -> sm (sxh);
                        # syt rows consumed above before being overwritten
                        tt(sm[:, :, 1:5, 1 : W - 1], f[:, :, :, 1 : W - 1], f[:, :, :, 1 : W - 1])
                        tt(sm[:, :, 1:5, 1 : W - 1], sm[:, :, 1:5, 1 : W - 1], f[:, :, :, 0 : W - 2])
                        tt(sm[:, :, 1:5, 1 : W - 1], sm[:, :, 1:5, 1 : W - 1], f[:, :, :, 2:W])
                        tt(sm[:, :, 1:5, 0:1], f[:, :, :, 0:1], f[:, :, :, 0:1])
                        tt(sm[:, :, 1:5, 0:1], sm[:, :, 1:5, 0:1], f[:, :, :, 0:1])
                        tt(sm[:, :, 1:5, 0:1], sm[:, :, 1:5, 0:1], f[:, :, :, 1:2])
                        tt(sm[:, :, 1:5, W - 1 : W], f[:, :, :, W - 1 : W], f[:, :, :, W - 1 : W])
                        tt(sm[:, :, 1:5, W - 1 : W], sm[:, :, 1:5, W - 1 : W], f[:, :, :, W - 1 : W])
                        tt(sm[:, :, 1:5, W - 1 : W], sm[:, :, 1:5, W - 1 : W], f[:, :, :, W - 2 : W - 1])
                        for rowdst, rowsrc in ((sm[:, :, 0:1, :], tN[:, :, 0:1, :]),
                                               (sm[:, :, 5:6, :], bN[:, :, 0:1, :])):
                            stt(v, rowdst[:, :, :, 1 : W - 1], rowsrc[:, :, :, 1 : W - 1],
                                rowsrc[:, :, :, 0 : W - 2], A.mult, A.add, 2.0)
                            stt(v, rowdst[:, :, :, 1 : W - 1], rowdst[:, :, :, 1 : W - 1],
                                rowsrc[:, :, :, 2:W], A.add, A.add)
                            stt(v, rowdst[:, :, :, 0:1], rowsrc[:, :, :, 0:1],
                                rowsrc[:, :, :, 1:2], A.mult, A.add, 3.0)
                            stt(v, rowdst[:, :, :, W - 1 : W], rowsrc[:, :, :, W - 1 : W],
                                rowsrc[:, :, :, W - 2 : W - 1], A.mult, A.add, 3.0)
                        tt(gy[:, :, :, :], sm[:, :, 2:6, :], sm[:, :, 0:4, :], A.subtract)
                        ayb = sm[:, :, 0:4, :]  # scratch: sm fully consumed by gy
                        s.activation(out=mg[:, :, :, :], in_=gx[:, :, :, :], func=F.Abs)
                        s.activation(out=ayb, in_=gy[:, :, :, :], func=F.Abs)
                        tt(mg[:, :, :, :], mg[:, :, :, :], ayb)
                        if c == 0:
                            v.tensor_copy(out=gxs[:, :, :, :], in_=gx[:, :, :, :])
                            v.tensor_copy(out=gys[:, :, :, :], in_=gy[:, :, :, :])
                            v.tensor_copy(out=mags[:, :, :, :], in_=mg[:, :, :, :])
                        else:
                            stt(v, sel[:, :, :, :], mg[:, :, :, :], mags[:, :, :, :],
                                A.bypass, A.is_gt)
                            v.copy_predicated(out=gxs[:, :, :, :], mask=sel[:, :, :, :], data=gx[:, :, :, :])
                            v.copy_predicated(out=gys[:, :, :, :], mask=sel[:, :, :, :], data=gy[:, :, :, :])
                            v.copy_predicated(out=mags[:, :, :, :], mask=sel[:, :, :, :], data=mg[:, :, :, :])
                pnum_cm.__exit__(None, None, None)  # free num

                # direction masks
                with tc.tile_pool(name="pdir", bufs=1, side="right") as pdir:
                    u = pdir.tile([128, NB, 4, W], F32, tag="u", name="u")
                    ax = pdir.tile([128, NB, 4, W], F32, tag="ax", name="ax")
                    tA = pdir.tile([128, NB, 4, W], F32, tag="tA", name="tA")
                    TA = pdir.tile([128, NB, 4, W], F32, tag="TA", name="TA")
                    t1 = pdir.tile([128, NB, 4, W], F32, tag="t1", name="t1")
                    b1 = pdir.tile([128, NB, 4, W], U8, tag="b1", name="b1")
                    b2 = pdir.tile([128, NB, 4, W], U8, tag="b2", name="b2")
                    v.tensor_scalar(ax[:, :, :, :], gxs[:, :, :, :], 0.0, 2.0, A.is_ge, A.mult)
                    stt(v, u[:, :, :, :], ax[:, :, :, :], gys[:, :, :, :], A.bypass, A.mult)
                    stt(v, u[:, :, :, :], u[:, :, :, :], gys[:, :, :, :], A.bypass, A.subtract)
                    s.activation(out=ax[:, :, :, :], in_=gxs[:, :, :, :], func=F.Abs)
                    v.tensor_scalar(tA[:, :, :, :], ax[:, :, :, :], T22, None, A.mult)
                    v.tensor_scalar(TA[:, :, :, :], ax[:, :, :, :], T67, None, A.mult)
                    # d0: -tA <= u < tA   (or mag == 0)
                    stt(v, t1[:, :, :, :], u[:, :, :, :], tA[:, :, :, :], A.bypass, A.add)
                    v.tensor_scalar(b1[:, :, :, :], t1[:, :, :, :], 0.0, None, A.is_ge)
                    stt(v, t1[:, :, :, :], u[:, :, :, :], tA[:, :, :, :], A.bypass, A.subtract)
                    v.tensor_scalar(b2[:, :, :, :], t1[:, :, :, :], 0.0, None, A.is_lt)
                    stt(v, d0m[:, :, :, :], b1[:, :, :, :], b2[:, :, :, :], A.bypass, A.logical_and)
                    v.tensor_scalar(b1[:, :, :, :], mags[:, :, :, :], 0.0, None, A.is_equal)
                    stt(v, d0m[:, :, :, :], d0m[:, :, :, :], b1[:, :, :, :], A.bypass, A.logical_or)
                    # d45: tA <= u < TA   (t1 still holds u - tA)
                    v.tensor_scalar(b1[:, :, :, :], t1[:, :, :, :], 0.0, None, A.is_ge)
                    stt(v, t1[:, :, :, :], u[:, :, :, :], TA[:, :, :, :], A.bypass, A.subtract)
                    v.tensor_scalar(b2[:, :, :, :], t1[:, :, :, :], 0.0, None, A.is_lt)
                    stt(v, d45[:, :, :, :], b1[:, :, :, :], b2[:, :, :, :], A.bypass, A.logical_and)
                    # d90: u >= TA or u < -TA   (t1 still holds u - TA)
                    v.tensor_scalar(b1[:, :, :, :], t1[:, :, :, :], 0.0, None, A.is_ge)
                    stt(v, t1[:, :, :, :], u[:, :, :, :], TA[:, :, :, :], A.bypass, A.add)
                    v.tensor_scalar(b2[:, :, :, :], t1[:, :, :, :], 0.0, None, A.is_lt)
                    stt(v, d90[:, :, :, :], b1[:, :, :, :], b2[:, :, :, :], A.bypass, A.logical_or)
                pgxy_cm.__exit__(None, None, None)  # free gxs/gys

                # neighbor picks + suppression (per image: copy_predicated
                # and its neighbor views must stay within 2 free dims)
                with tc.tile_pool(name="pnbr", bufs=1, side="right") as pnbr:
                    mh = pnbr.tile([128, 6, WH], F32, tag="mh", name="mh")
                    n1 = pnbr.tile([128, 4, W], F32, tag="n1", name="n1")
                    n2 = pnbr.tile([128, 4, W], F32, tag="n2", name="n2")
                    for b in range(NB):
                        mb = mags[:, b, :, :]
                        v.memset(mh[:, :, :], 0.0)
                        v.tensor_copy(out=mh[:, 1:5, 1 : 1 + W], in_=mb)
                        nc.sync.dma_start(out=mh[1:128, 0:1, 1 : 1 + W], in_=mb[0:127, 3:4, :])
                        nc.sync.dma_start(out=mh[0:127, 5:6, 1 : 1 + W], in_=mb[1:128, 0:1, :])

                        def nbr(dy2, dx2):
                            return mh[:, 1 + dy2 : 5 + dy2, 1 + dx2 : 1 + dx2 + W]

                        v.tensor_copy(out=n1[:, :, :], in_=nbr(-1, -1))
                        v.copy_predicated(out=n1[:, :, :], mask=d90[:, b, :, :], data=nbr(-1, 0))
                        v.copy_predicated(out=n1[:, :, :], mask=d45[:, b, :, :], data=nbr(-1, 1))
                        v.copy_predicated(out=n1[:, :, :], mask=d0m[:, b, :, :], data=nbr(0, 1))
                        v.tensor_copy(out=n2[:, :, :], in_=nbr(1, 1))
                        v.copy_predicated(out=n2[:, :, :], mask=d90[:, b, :, :], data=nbr(1, 0))
                        v.copy_predicated(out=n2[:, :, :], mask=d45[:, b, :, :], data=nbr(1, -1))
                        v.copy_predicated(out=n2[:, :, :], mask=d0m[:, b, :, :], data=nbr(0, -1))
                        stt(v, n1[:, :, :], mb, n1[:, :, :], A.bypass, A.is_ge)
                        stt(v, n2[:, :, :], mb, n2[:, :, :], A.bypass, A.is_ge)
                        stt(v, n1[:, :, :], n1[:, :, :], n2[:, :, :], A.bypass, A.mult)
                        stt(v, nmst[:, b, :, :], mb, n1[:, :, :], A.bypass, A.mult)

            # ---------- hysteresis (both images batched) ----------
            with tc.tile_pool(name="phy", bufs=1) as phy:
                st = phy.tile([128, NB, 4, WH], F16, tag="st", name="st")
                sc = phy.tile([128, NB, 4, WH], F16, tag="sc", name="sc")
                wk = phy.tile([128, NB, 4, WH], F16, tag="wk", name="wk")
                hdil = phy.tile([128, NB, 4, WH], F16, tag="hdil", name="hdil")
                vdil = phy.tile([128, NB, 6, WH], F16, tag="vdil", name="vdil")
                dil = phy.tile([128, NB, 4, WH], F16, tag="dil", name="dil")
                v.memset(st[:, :, :, :], 0.0)
                v.memset(wk[:, :, :, :], 0.0)
                v.memset(hdil[:, :, :, :], 0.0)
                v.memset(vdil[:, :, :, :], 0.0)
                if HYST_INIT_TT:
                    hbc = nc.const_aps.tensor(HIGH_T, (128, NB, 4, W))
                    lbc = nc.const_aps.tensor(LOW_T, (128, NB, 4, W))
                    v.tensor_tensor(out=st[:, :, :, 1 : 1 + W], in0=nmst[:, :, :, :],
                                    in1=hbc, op=A.is_gt)
                    v.tensor_tensor(out=wk[:, :, :, 1 : 1 + W], in0=nmst[:, :, :, :],
                                    in1=lbc, op=A.is_gt)
                else:
                    thr = phy.tile([128, NB, 4, W], F16, tag="thr", name="thr")
                    v.tensor_scalar(thr[:, :, :, :], nmst[:, :, :, :], HIGH_T, None, A.is_gt)
                    nc.sync.dma_start(out=st[:, :, :, 1 : 1 + W], in_=thr[:, :, :, :])
                    v.tensor_scalar(thr[:, :, :, :], nmst[:, :, :, :], LOW_T, None, A.is_gt)
                    nc.sync.dma_start(out=wk[:, :, :, 1 : 1 + W], in_=thr[:, :, :, :])
                st2d = st[:, :, :, :].rearrange("p i a x -> p (i a x)")
                sc2d = sc[:, :, :, :].rearrange("p i a x -> p (i a x)")
                wk2d = wk[:, :, :, :].rearrange("p i a x -> p (i a x)")
                for _it in range(HYST_ITERS):
                    v.tensor_tensor_scan(out=sc2d, data0=wk2d, data1=st2d,
                                         initial=0.0, op0=A.mult, op1=A.max)
                    v.tensor_tensor_scan(out=st2d[:, ::-1], data0=wk2d[:, ::-1],
                                         data1=sc2d[:, ::-1], initial=0.0, op0=A.mult, op1=A.max)
                    v.tensor_tensor(out=hdil[:, :, :, 1 : 1 + W], in0=st[:, :, :, 0:W],
                                    in1=st[:, :, :, 2 : 2 + W], op=A.max)
                    v.tensor_tensor(out=vdil[:, :, 1:5, 1 : 1 + W], in0=hdil[:, :, :, 1 : 1 + W],
                                    in1=st[:, :, :, 1 : 1 + W], op=A.max)
                    exch = _it % 2 == 0 if HYST_SKIP_LAST_X else True
                    if exch:
                        nc.sync.dma_start(out=vdil[1:128, :, 0:1, 1 : 1 + W], in_=vdil[0:127, :, 4:5, 1 : 1 + W])
                        nc.sync.dma_start(out=vdil[0:127, :, 5:6, 1 : 1 + W], in_=vdil[1:128, :, 1:2, 1 : 1 + W])
                    # middle output rows (1,2) need only vdil rows 1..4: they
                    # run while the exchange DMAs are in flight
                    stt(v, dil[:, :, 1:3, :], vdil[:, :, 1:3, :], vdil[:, :, 3:5, :], A.bypass, A.max)
                    stt(v, dil[:, :, 1:3, :], dil[:, :, 1:3, :], vdil[:, :, 2:4, :], A.bypass, A.max)
                    if exch:
                        stt(v, dil[:, :, 0:1, :], vdil[:, :, 0:1, :], vdil[:, :, 2:3, :], A.bypass, A.max)
                        stt(v, dil[:, :, 0:1, :], dil[:, :, 0:1, :], vdil[:, :, 1:2, :], A.bypass, A.max)
                        stt(v, dil[:, :, 3:4, :], vdil[:, :, 3:4, :], vdil[:, :, 5:6, :], A.bypass, A.max)
                        stt(v, dil[:, :, 3:4, :], dil[:, :, 3:4, :], vdil[:, :, 4:5, :], A.bypass, A.max)
                    else:
                        stt(v, dil[:, :, 0:1, :], vdil[:, :, 1:2, :], vdil[:, :, 2:3, :], A.bypass, A.max)
                        stt(v, dil[:, :, 3:4, :], vdil[:, :, 3:4, :], vdil[:, :, 4:5, :], A.bypass, A.max)
                    stt(v, dil[:, :, :, :], dil[:, :, :, :], wk[:, :, :, :], A.bypass, A.mult)
                    stt(v, st[:, :, :, :], st[:, :, :, :], dil[:, :, :, :], A.bypass, A.max)
                o32 = phy.tile([128, NB, 4, W], F32, tag="o32", name="o32")
                v.tensor_tensor(out=o32[:, :, :, :], in0=st[:, :, :, 1 : 1 + W],
                                in1=st[:, :, :, 1 : 1 + W], op=A.max)
                nc.sync.dma_start(
                    out=out[:, :, :].rearrange("b (p r) x -> p b r x", r=4),
                    in_=o32[:, :, :, :],
                )
    nc.finalize()
    return nc


# ---------------------------------------------------------------------------
# walrus 1-sync-wait-per-instruction workaround (BIR JSON post-pass)
# ---------------------------------------------------------------------------
import json as _json

_ws_counter = [0]


def _split_instruction_list(instrs):
    out = []
    for ins in instrs:
        si = ins.get("sync_info")
        waits = (si or {}).get("on_wait") or []
        if len(waits) > 1:
            for wcond in waits[:-1]:
                _ws_counter[0] += 1
                out.append({
                    "debug": ins.get("debug", 0),
                    "engine": ins["engine"],
                    "ins": [],
                    "name": f"I-waitsplit-{_ws_counter[0]}",
                    "opcode": "NoOp",
                    "outs": [],
                    "sync_info": {"on_wait": [wcond], "on_update": []},
                })
            si = dict(si)
            si["on_wait"] = [waits[-1]]
            ins = dict(ins)
            ins["sync_info"] = si
        out.append(ins)
    return out


def _walk_split(obj):
    if isinstance(obj, dict):
        for k, val in obj.items():
            if k == "instructions" and isinstance(val, list):
                obj[k] = _split_instruction_list(val)
            else:
                _walk_split(val)
    elif isinstance(obj, list):
        for val in obj:
            _walk_split(val)


def _split_multiwait_bir(bir_json):
    j = _json.loads(bir_json)
    _walk_split(j)
    return _json.dumps(j).encode()


_patched = [False]


def _install_bir_patch():
    if _patched[0]:
        return
    _patched[0] = True
    import concourse.bass_utils as bu

    orig = bu.compile_bir_kernel

    def patched(bir_json, tmpdir, neff_name="file.neff"):
        return orig(_split_multiwait_bir(bir_json), tmpdir, neff_name)

    bu.compile_bir_kernel = patched
    try:
        import concourse.bass2jax as b2j

        b2j.compile_bir_kernel = patched
    except Exception:
        pass


# ---------------------------------------------------------------------------
# host entry point
# ---------------------------------------------------------------------------
_cache = {}


def _get_program(rep=1):
    key = ("nc", rep)
    if key not in _cache:
        _install_bir_patch()
        _cache[key] = build(rep=rep)
    return _cache[key]


def make_in_maps(x):
    x = np.asarray(x, dtype=np.float32)
    xpad = np.pad(x, ((0, 0), (0, 0), (PAD, PAD), (PAD, PAD)), mode="reflect")
    return [
        {"xp": np.ascontiguousarray(xpad[i * NB : (i + 1) * NB])}
        for i in range(NCORES)
    ]


def kernel(x):
    """x: [16,3,512,512] float32 -> edges [16,1,512,512] float32."""
    from concourse.bass_utils import run_bass_kernel_spmd

    x = np.asarray(x, dtype=np.float32)
    B = x.shape[0]
    assert x.shape == (NCORES * NB, 3, H, W), x.shape
    nc = _get_program()
    in_maps = make_in_maps(x)
    res = run_bass_kernel_spmd(nc, in_maps, core_ids=list(range(NCORES)))
    out = np.empty((B, 1, H, W), np.float32)
    for i in range(NCORES):
        out[i * NB : (i + 1) * NB, 0] = res.results[i]["edges"]
    return out


# revision 27
# speedup vs baseline: 4.3544x; 1.0676x over previous
"""Trainium2 Bass kernel for nn_BilateralModule (bilateral filter + Canny
NMS + hysteresis), data-parallel across 8 NeuronCores (2 images per core).

Fully unrolled design (no runtime For_i loops except the benchmark rep
loop): the terminal charges ~engine-time + small per-instruction overhead
for straight-line code, while For_i iterations carry a large per-iteration
sync storm (drains + semaphores on all five engines). The bilateral uses
the exact 49-tap circular window with static spatial weights folded into
per-tap immediates, f16 compute in the DVE 4x path (TensorScalarPtr ops),
and splits work across DVE (diff/square/products/accumulate), Act
(|.|, exp) and Pool (channel sums, den accumulate).

Layout: partition p holds image rows 4p..4p+3; dim1 fuses (channel, image)
c*NB+b so engine views stay within 3 free dims.

Also includes a workaround for this walrus build accepting at most ONE
sync-wait per instruction: extra waits are hoisted onto NoOps inserted just
before the instruction on the same engine (identical program-order
semantics).
"""
import numpy as np

import concourse.bass as bass
import concourse.bacc as bacc
import concourse.mybir as mybir
from concourse.mybir import AluOpType as A, ActivationFunctionType as F
from concourse.tile import TileContext

F32 = mybir.dt.float32
F16 = mybir.dt.float16
U8 = mybir.dt.uint8

H = W = 512
PAD = 4
WP = W + 2 * PAD  # 520
WH = W + 2  # 514
MAGIC = 12582912.0
GC = -0.5 / 75.0 ** 2
CS = 32.0  # |diff| pre-scale so cd^2 stays in f16 range
GCS = GC * CS * CS
HIGH_T = 150.0
LOW_T = 50.0
T22 = float(np.tan(np.radians(22.5)))
T67 = float(np.tan(np.radians(67.5)))
HYST_ITERS = 4
HYST_INIT_TT = True     # init st/wk via const-broadcast TT (False: ts+DMA)
HYST_SKIP_LAST_X = True  # skip cross-partition exchange on last iter
NB = 2
NCORES = 8
C6 = 3 * NB

TAPS = [
    (dy, dx)
    for dy in range(-PAD, PAD + 1)
    for dx in range(-PAD, PAD + 1)
    if 0 < dy * dy + dx * dx <= PAD * PAD
]


def build(rep=1):
    nc = bacc.Bacc()
    xp = nc.dram_tensor("xp", [NB, 3, WP, WP], F32, kind="ExternalInput")
    out = nc.dram_tensor("edges", [NB, H, W], F32, kind="ExternalOutput")
    e16d = nc.dram_tensor("e16d", [C6, 8, WP], F16)
    v = nc.vector
    s = nc.scalar
    g = nc.gpsimd

    def stt(eng, out, in0, in1, op0, op1, scalar=0.0):
        eng.scalar_tensor_tensor(out=out, in0=in0, scalar=scalar, in1=in1,
                                 op0=op0, op1=op1)

    # activation float biases / threshold tensors need pre-registered consts
    cvals = [float(GC * r2v) for r2v in sorted({dy * dy + dx * dx for dy, dx in TAPS})]
    cvals += [HIGH_T, LOW_T]
    for i, val in enumerate(cvals):
        t = nc.alloc_sbuf_tensor(f"const-k{i}", [128, 1], F32)
        nc.gpsimd.memset(t.ap(), val)
        nc.const_aps.aps[(F32, val)] = t.ap()

    with TileContext(nc) as tc:
        with tc.For_i(0, rep, 1) as _r:
            pnum_cm = tc.tile_pool(name="pnum", bufs=1)
            pnum = pnum_cm.__enter__()
            num = pnum.tile([128, C6, 4, W], F32, tag="num", name="num")

            pimg_cm = tc.tile_pool(name="pimg", bufs=1)
            pimg = pimg_cm.__enter__()
            imgA = pimg.tile([128, C6, 12, WP], F16, tag="imgA", name="imgA")

            # ---------- load + quantize straight into imgA rows 0:4 ----
            with tc.tile_pool(name="pq", bufs=1) as pq:
                qs = pq.tile([128, NB, 3, 4, WP], F32, tag="qs", name="qs")
                qe = pq.tile([8, NB, 3, 1, WP], F32, tag="qe", name="qe")
                e16 = pq.tile([8, C6, 1, WP], F16, tag="e16", name="e16")
                # bottom-rows chain first so its DRAM round trip overlaps
                # the big quantize below
                nc.sync.dma_start(
                    out=qe[:, :, :, :, :],
                    in_=xp[:, :, H : H + 8, :].rearrange("b c (p r) x -> p b c r x", r=1),
                )
                v.tensor_scalar(qe[:, :, :, :, :], qe[:, :, :, :, :], 0.0, 1.0, A.max, A.min)
                v.tensor_scalar(qe[:, :, :, :, :], qe[:, :, :, :, :], 255.0, MAGIC, A.mult, A.add)
                e16v = e16[:, :, :, :].rearrange("p (b c) r x -> p b c r x", c=3)
                v.tensor_scalar(e16v, qe[:, :, :, :, :], MAGIC, None, A.subtract)
                nc.sync.dma_start(out=e16d[:, :, :].rearrange("c r x -> r c x"),
                                  in_=e16[:, :, 0, :])
                nc.sync.dma_start(out=imgA[127:128, :, 4:8, :], in_=e16d[:, 0:4, :])
                nc.sync.dma_start(out=imgA[126:127, :, 8:12, :], in_=e16d[:, 0:4, :])
                nc.sync.dma_start(out=imgA[127:128, :, 8:12, :], in_=e16d[:, 4:8, :])
                src = xp[:, :, 0:H, :].rearrange("b c (p r) x -> p b c r x", r=4)
                nc.sync.dma_start(out=qs[:, :, :, :, :], in_=src)
                v.tensor_scalar(qs[:, :, :, :, :], qs[:, :, :, :, :], 0.0, 1.0, A.max, A.min)
                v.tensor_scalar(qs[:, :, :, :, :], qs[:, :, :, :, :], 255.0, MAGIC, A.mult, A.add)
                dst = imgA[:, :, 0:4, :].rearrange("p (b c) r x -> p b c r x", c=3)
                v.tensor_scalar(dst, qs[:, :, :, :, :], MAGIC, None, A.subtract)
                nc.sync.dma_start(out=imgA[0:127, :, 4:8, :], in_=imgA[1:128, :, 0:4, :])
                nc.sync.dma_start(out=imgA[0:126, :, 8:12, :], in_=imgA[2:128, :, 0:4, :])

            # ---------- bilateral: 48 unrolled taps + center ----------
            # f16 only where values are exact integers (image, |diff|, channel
            # sums <= 765); everything from Square onward is f32.
            ctr6 = imgA[:, :, 4:8, 4 : 4 + W]
            v.tensor_tensor(out=num[:, :, :, :], in0=ctr6, in1=ctr6, op=A.max)
            pt_cm = tc.tile_pool(name="pt", bufs=1)
            pt = pt_cm.__enter__()
            if True:
                den = pt.tile([128, NB, 4, W], F32, tag="den", name="den")
                g.memset(den[:, :, :, :], 1.0)
                dt_ = pt.tile([128, C6, 4, W], F16, tag="dt", name="dt")
                cds = pt.tile([128, NB, 4, W], F16, tag="cds", name="cds")
                sqw = pt.tile([128, NB, 4, W], F32, tag="sqw", name="sqw")
                pr = pt.tile([128, NB, 4, W], F32, tag="pr", name="pr")
                for dy, dx in TAPS:
                    sy, sx = dy + PAD, dx + PAD
                    bias_r2 = float(GC * (dy * dy + dx * dx))
                    sh6 = imgA[:, :, sy : sy + 4, sx : sx + W]
                    v.tensor_tensor(out=dt_[:, :, :, :], in0=sh6, in1=ctr6, op=A.subtract)
                    s.activation(out=dt_[:, :, :, :], in_=dt_[:, :, :, :], func=F.Abs)
                    g.tensor_tensor(out=cds[:, :, :, :], in0=dt_[:, 0::3, :, :],
                                    in1=dt_[:, 1::3, :, :], op=A.add)
                    g.tensor_tensor(out=cds[:, :, :, :], in0=cds[:, :, :, :],
                                    in1=dt_[:, 2::3, :, :], op=A.add)
                    s.activation(out=sqw[:, :, :, :], in_=cds[:, :, :, :], func=F.Square)
                    # wt = exp(GC*cd^2 + GC*r2): full bilateral weight
                    s.activation(out=sqw[:, :, :, :], in_=sqw[:, :, :, :], func=F.Exp,
                                 scale=GC, bias=bias_r2)
                    for c in range(3):
                        sh_c = imgA[:, c::3, sy : sy + 4, sx : sx + W]
                        v.tensor_tensor(out=pr[:, :, :, :], in0=sqw[:, :, :, :],
                                        in1=sh_c, op=A.mult)
                        v.tensor_tensor(out=num[:, c::3, :, :], in0=num[:, c::3, :, :],
                                        in1=pr[:, :, :, :], op=A.add)
                    g.tensor_tensor(out=den[:, :, :, :], in0=sqw[:, :, :, :],
                                    in1=den[:, :, :, :], op=A.add)
            # ---------- divide in place: num becomes filt (f32) ----------
            rcp = pr  # pr is dead after the last tap
            v.reciprocal(out=rcp[:, :, :, :], in_=den[:, :, :, :])
            for c in range(3):
                v.tensor_tensor(out=num[:, c::3, :, :], in0=num[:, c::3, :, :],
                                in1=rcp[:, :, :, :], op=A.mult)
            pt_cm.__exit__(None, None, None)
            pimg_cm.__exit__(None, None, None)  # free imgA

            # ---------- NMS: channel-sequential, both images batched ------
            with tc.tile_pool(name="psel", bufs=1, side="right") as psel:
                mags = psel.tile([128, NB, 4, W], F32, tag="mags", name="mags")
                nmst = psel.tile([128, NB, 4, W], F32, tag="nmst", name="nmst")
                d0m = psel.tile([128, NB, 4, W], U8, tag="d0m", name="d0m")
                d45 = psel.tile([128, NB, 4, W], U8, tag="d45", name="d45")
                d90 = psel.tile([128, NB, 4, W], U8, tag="d90", name="d90")
                pgxy_cm = tc.tile_pool(name="pgxy", bufs=1, side="right")
                pgxy = pgxy_cm.__enter__()
                gxs = pgxy.tile([128, NB, 4, W], F32, tag="gxs", name="gxs")
                gys = pgxy.tile([128, NB, 4, W], F32, tag="gys", name="gys")
                with tc.tile_pool(name="pch", bufs=1, side="right") as pch:
                    tN = pch.tile([128, NB, 1, W], F32, tag="tN", name="tN")
                    bN = pch.tile([128, NB, 1, W], F32, tag="bN", name="bN")
                    sm = pch.tile([128, NB, 6, W], F32, tag="sm", name="sm")
                    gx = pch.tile([128, NB, 4, W], F32, tag="gx", name="gx")
                    gy = pch.tile([128, NB, 4, W], F32, tag="gy", name="gy")
                    mg = pch.tile([128, NB, 4, W], F32, tag="mg", name="mg")
                    sel = d0m  # scratch reuse: d0m is only written in the dir phase

                    def tt(out_, a, b_, op=A.add):
                        v.tensor_tensor(out=out_, in0=a, in1=b_, op=op)

                    for c in range(3):
                        f = num[:, c::3, :, :]
                        v.tensor_copy(out=tN[:, :, 0, :], in_=f[:, :, 0, :])
                        v.tensor_copy(out=bN[:, :, 0, :], in_=f[:, :, 3, :])
                        nc.sync.dma_start(out=tN[1:128, :, 0, :], in_=f[0:127, :, 3, :])
                        nc.sync.dma_start(out=bN[0:127, :, 0, :], in_=f[1:128, :, 0, :])
                        # vertical 1-2-1 -> sm rows 0:4 (syt): 2f + up + down
                        tt(sm[:, :, 1:3, :], f[:, :, 1:3, :], f[:, :, 1:3, :])
                        tt(sm[:, :, 1:3, :], sm[:, :, 1:3, :], f[:, :, 0:2, :])
                        tt(sm[:, :, 1:3, :], sm[:, :, 1:3, :], f[:, :, 2:4, :])
                        tt(sm[:, :, 0:1, :], f[:, :, 0:1, :], f[:, :, 0:1, :])
                        tt(sm[:, :, 0:1, :], sm[:, :, 0:1, :], tN[:, :, 0:1, :])
                        tt(sm[:, :, 0:1, :], sm[:, :, 0:1, :], f[:, :, 1:2, :])
                        tt(sm[:, :, 3:4, :], f[:, :, 3:4, :], f[:, :, 3:4, :])
                        tt(sm[:, :, 3:4, :], sm[:, :, 3:4, :], f[:, :, 2:3, :])
                        tt(sm[:, :, 3:4, :], sm[:, :, 3:4, :], bN[:, :, 0:1, :])
                        syt = sm[:, :, 0:4, :]
                        tt(gx[:, :, :, 1 : W - 1], syt[:, :, :, 2:W], syt[:, :, :, 0 : W - 2], A.subtract)
                        tt(gx[:, :, :, 0:1], syt[:, :, :, 1:2], syt[:, :, :, 0:1], A.subtract)
                        tt(gx[:, :, :, W - 1 : W], syt[:, :, :, W - 1 : W], syt[:, :, :, W - 2 : W - 1], A.subtract)
                        # horizontal 1-2-1 over 6 virtual rows -> sm (sxh);
                        # syt rows consumed above before being overwritten
                        tt(sm[:, :, 1:5, 1 : W - 1], f[:, :, :, 1 : W - 1], f[:, :, :, 1 : W - 1])
                        tt(sm[:, :, 1:5, 1 : W - 1], sm[:, :, 1:5, 1 : W - 1], f[:, :, :, 0 : W - 2])
                        tt(sm[:, :, 1:5, 1 : W - 1], sm[:, :, 1:5, 1 : W - 1], f[:, :, :, 2:W])
                        tt(sm[:, :, 1:5, 0:1], f[:, :, :, 0:1], f[:, :, :, 0:1])
                        tt(sm[:, :, 1:5, 0:1], sm[:, :, 1:5, 0:1], f[:, :, :, 0:1])
                        tt(sm[:, :, 1:5, 0:1], sm[:, :, 1:5, 0:1], f[:, :, :, 1:2])
                        tt(sm[:, :, 1:5, W - 1 : W], f[:, :, :, W - 1 : W], f[:, :, :, W - 1 : W])
                        tt(sm[:, :, 1:5, W - 1 : W], sm[:, :, 1:5, W - 1 : W], f[:, :, :, W - 1 : W])
                        tt(sm[:, :, 1:5, W - 1 : W], sm[:, :, 1:5, W - 1 : W], f[:, :, :, W - 2 : W - 1])
                        for rowdst, rowsrc in ((sm[:, :, 0:1, :], tN[:, :, 0:1, :]),
                                               (sm[:, :, 5:6, :], bN[:, :, 0:1, :])):
                            stt(v, rowdst[:, :, :, 1 : W - 1], rowsrc[:, :, :, 1 : W - 1],
                                rowsrc[:, :, :, 0 : W - 2], A.mult, A.add, 2.0)
                            stt(v, rowdst[:, :, :, 1 : W - 1], rowdst[:, :, :, 1 : W - 1],
                                rowsrc[:, :, :, 2:W], A.add, A.add)
                            stt(v, rowdst[:, :, :, 0:1], rowsrc[:, :, :, 0:1],
                                rowsrc[:, :, :, 1:2], A.mult, A.add, 3.0)
                            stt(v, rowdst[:, :, :, W - 1 : W], rowsrc[:, :, :, W - 1 : W],
                                rowsrc[:, :, :, W - 2 : W - 1], A.mult, A.add, 3.0)
                        tt(gy[:, :, :, :], sm[:, :, 2:6, :], sm[:, :, 0:4, :], A.subtract)
                        ayb = sm[:, :, 0:4, :]  # scratch: sm fully consumed by gy
                        s.activation(out=mg[:, :, :, :], in_=gx[:, :, :, :], func=F.Abs)
                        s.activation(out=ayb, in_=gy[:, :, :, :], func=F.Abs)
                        tt(mg[:, :, :, :], mg[:, :, :, :], ayb)
                        if c == 0:
                            v.tensor_copy(out=gxs[:, :, :, :], in_=gx[:, :, :, :])
                            v.tensor_copy(out=gys[:, :, :, :], in_=gy[:, :, :, :])
                            v.tensor_copy(out=mags[:, :, :, :], in_=mg[:, :, :, :])
                        else:
                            stt(v, sel[:, :, :, :], mg[:, :, :, :], mags[:, :, :, :],
                                A.bypass, A.is_gt)
                            v.copy_predicated(out=gxs[:, :, :, :], mask=sel[:, :, :, :], data=gx[:, :, :, :])
                            v.copy_predicated(out=gys[:, :, :, :], mask=sel[:, :, :, :], data=gy[:, :, :, :])
                            v.copy_predicated(out=mags[:, :, :, :], mask=sel[:, :, :, :], data=mg[:, :, :, :])
                pnum_cm.__exit__(None, None, None)  # free num

                # direction masks
                with tc.tile_pool(name="pdir", bufs=1, side="right") as pdir:
                    u = pdir.tile([128, NB, 4, W], F32, tag="u", name="u")
                    ax = pdir.tile([128, NB, 4, W], F32, tag="ax", name="ax")
                    tA = pdir.tile([128, NB, 4, W], F32, tag="tA", name="tA")
                    TA = pdir.tile([128, NB, 4, W], F32, tag="TA", name="TA")
                    t1 = pdir.tile([128, NB, 4, W], F32, tag="t1", name="t1")
                    b1 = pdir.tile([128, NB, 4, W], U8, tag="b1", name="b1")
                    b2 = pdir.tile([128, NB, 4, W], U8, tag="b2", name="b2")
                    v.tensor_scalar(ax[:, :, :, :], gxs[:, :, :, :], 0.0, 2.0, A.is_ge, A.mult)
                    stt(v, u[:, :, :, :], ax[:, :, :, :], gys[:, :, :, :], A.bypass, A.mult)
                    stt(v, u[:, :, :, :], u[:, :, :, :], gys[:, :, :, :], A.bypass, A.subtract)
                    s.activation(out=ax[:, :, :, :], in_=gxs[:, :, :, :], func=F.Abs)
                    v.tensor_scalar(tA[:, :, :, :], ax[:, :, :, :], T22, None, A.mult)
                    v.tensor_scalar(TA[:, :, :, :], ax[:, :, :, :], T67, None, A.mult)
                    # d0: -tA <= u < tA   (or mag == 0)
                    stt(v, t1[:, :, :, :], u[:, :, :, :], tA[:, :, :, :], A.bypass, A.add)
                    v.tensor_scalar(b1[:, :, :, :], t1[:, :, :, :], 0.0, None, A.is_ge)
                    stt(v, t1[:, :, :, :], u[:, :, :, :], tA[:, :, :, :], A.bypass, A.subtract)
                    v.tensor_scalar(b2[:, :, :, :], t1[:, :, :, :], 0.0, None, A.is_lt)
                    stt(v, d0m[:, :, :, :], b1[:, :, :, :], b2[:, :, :, :], A.bypass, A.logical_and)
                    v.tensor_scalar(b1[:, :, :, :], mags[:, :, :, :], 0.0, None, A.is_equal)
                    stt(v, d0m[:, :, :, :], d0m[:, :, :, :], b1[:, :, :, :], A.bypass, A.logical_or)
                    # d45: tA <= u < TA   (t1 still holds u - tA)
                    v.tensor_scalar(b1[:, :, :, :], t1[:, :, :, :], 0.0, None, A.is_ge)
                    stt(v, t1[:, :, :, :], u[:, :, :, :], TA[:, :, :, :], A.bypass, A.subtract)
                    v.tensor_scalar(b2[:, :, :, :], t1[:, :, :, :], 0.0, None, A.is_lt)
                    stt(v, d45[:, :, :, :], b1[:, :, :, :], b2[:, :, :, :], A.bypass, A.logical_and)
                    # d90: u >= TA or u < -TA   (t1 still holds u - TA)
                    v.tensor_scalar(b1[:, :, :, :], t1[:, :, :, :], 0.0, None, A.is_ge)
                    stt(v, t1[:, :, :, :], u[:, :, :, :], TA[:, :, :, :], A.bypass, A.add)
                    v.tensor_scalar(b2[:, :, :, :], t1[:, :, :, :], 0.0, None, A.is_lt)
                    stt(v, d90[:, :, :, :], b1[:, :, :, :], b2[:, :, :, :], A.bypass, A.logical_or)
                pgxy_cm.__exit__(None, None, None)  # free gxs/gys

                # neighbor picks + suppression (per image: copy_predicated
                # and its neighbor views must stay within 2 free dims)
                with tc.tile_pool(name="pnbr", bufs=1, side="right") as pnbr:
                    mh = pnbr.tile([128, 6, WH], F32, tag="mh", name="mh")
                    n1 = pnbr.tile([128, 4, W], F32, tag="n1", name="n1")
                    n2 = pnbr.tile([128, 4, W], F32, tag="n2", name="n2")
                    for b in range(NB):
                        mb = mags[:, b, :, :]
                        v.memset(mh[:, :, :], 0.0)
                        v.tensor_copy(out=mh[:, 1:5, 1 : 1 + W], in_=mb)
                        nc.sync.dma_start(out=mh[1:128, 0:1, 1 : 1 + W], in_=mb[0:127, 3:4, :])
                        nc.sync.dma_start(out=mh[0:127, 5:6, 1 : 1 + W], in_=mb[1:128, 0:1, :])

                        def nbr(dy2, dx2):
                            return mh[:, 1 + dy2 : 5 + dy2, 1 + dx2 : 1 + dx2 + W]

                        v.tensor_copy(out=n1[:, :, :], in_=nbr(-1, -1))
                        v.copy_predicated(out=n1[:, :, :], mask=d90[:, b, :, :], data=nbr(-1, 0))
                        v.copy_predicated(out=n1[:, :, :], mask=d45[:, b, :, :], data=nbr(-1, 1))
                        v.copy_predicated(out=n1[:, :, :], mask=d0m[:, b, :, :], data=nbr(0, 1))
                        v.tensor_copy(out=n2[:, :, :], in_=nbr(1, 1))
                        v.copy_predicated(out=n2[:, :, :], mask=d90[:, b, :, :], data=nbr(1, 0))
                        v.copy_predicated(out=n2[:, :, :], mask=d45[:, b, :, :], data=nbr(1, -1))
                        v.copy_predicated(out=n2[:, :, :], mask=d0m[:, b, :, :], data=nbr(0, -1))
                        stt(v, n1[:, :, :], mb, n1[:, :, :], A.bypass, A.is_ge)
                        stt(v, n2[:, :, :], mb, n2[:, :, :], A.bypass, A.is_ge)
                        stt(v, n1[:, :, :], n1[:, :, :], n2[:, :, :], A.bypass, A.mult)
                        stt(v, nmst[:, b, :, :], mb, n1[:, :, :], A.bypass, A.mult)

            # ---------- hysteresis (both images batched) ----------
            with tc.tile_pool(name="phy", bufs=1) as phy:
                st = phy.tile([128, NB, 4, WH], F16, tag="st", name="st")
                sc = phy.tile([128, NB, 4, WH], F16, tag="sc", name="sc")
                wk = phy.tile([128, NB, 4, WH], F16, tag="wk", name="wk")
                hdil = phy.tile([128, NB, 4, WH], F16, tag="hdil", name="hdil")
                vdil = phy.tile([128, NB, 6, WH], F16, tag="vdil", name="vdil")
                dil = phy.tile([128, NB, 4, WH], F16, tag="dil", name="dil")
                v.memset(st[:, :, :, :], 0.0)
                v.memset(wk[:, :, :, :], 0.0)
                v.memset(hdil[:, :, :, :], 0.0)
                v.memset(vdil[:, :, :, :], 0.0)
                if HYST_INIT_TT:
                    hbc = nc.const_aps.tensor(HIGH_T, (128, NB, 4, W))
                    lbc = nc.const_aps.tensor(LOW_T, (128, NB, 4, W))
                    v.tensor_tensor(out=st[:, :, :, 1 : 1 + W], in0=nmst[:, :, :, :],
                                    in1=hbc, op=A.is_gt)
                    v.tensor_tensor(out=wk[:, :, :, 1 : 1 + W], in0=nmst[:, :, :, :],
                                    in1=lbc, op=A.is_gt)
                else:
                    thr = phy.tile([128, NB, 4, W], F16, tag="thr", name="thr")
                    v.tensor_scalar(thr[:, :, :, :], nmst[:, :, :, :], HIGH_T, None, A.is_gt)
                    nc.sync.dma_start(out=st[:, :, :, 1 : 1 + W], in_=thr[:, :, :, :])
                    v.tensor_scalar(thr[:, :, :, :], nmst[:, :, :, :], LOW_T, None, A.is_gt)
                    nc.sync.dma_start(out=wk[:, :, :, 1 : 1 + W], in_=thr[:, :, :, :])
                st2d = st[:, :, :, :].rearrange("p i a x -> p (i a x)")
                sc2d = sc[:, :, :, :].rearrange("p i a x -> p (i a x)")
                wk2d = wk[:, :, :, :].rearrange("p i a x -> p (i a x)")
                for _it in range(HYST_ITERS):
                    v.tensor_tensor_scan(out=sc2d, data0=wk2d, data1=st2d,
                                         initial=0.0, op0=A.mult, op1=A.max)
                    v.tensor_tensor_scan(out=st2d[:, ::-1], data0=wk2d[:, ::-1],
                                         data1=sc2d[:, ::-1], initial=0.0, op0=A.mult, op1=A.max)
                    v.tensor_tensor(out=hdil[:, :, :, 1 : 1 + W], in0=st[:, :, :, 0:W],
                                    in1=st[:, :, :, 2 : 2 + W], op=A.max)
                    v.tensor_tensor(out=vdil[:, :, 1:5, 1 : 1 + W], in0=hdil[:, :, :, 1 : 1 + W],
                                    in1=st[:, :, :, 1 : 1 + W], op=A.max)
                    exch = _it % 2 == 0 if HYST_SKIP_LAST_X else True
                    if exch:
                        nc.sync.dma_start(out=vdil[1:128, :, 0:1, 1 : 1 + W], in_=vdil[0:127, :, 4:5, 1 : 1 + W])
                        nc.sync.dma_start(out=vdil[0:127, :, 5:6, 1 : 1 + W], in_=vdil[1:128, :, 1:2, 1 : 1 + W])
                    # middle output rows (1,2) need only vdil rows 1..4: they
                    # run while the exchange DMAs are in flight
                    stt(v, dil[:, :, 1:3, :], vdil[:, :, 1:3, :], vdil[:, :, 3:5, :], A.bypass, A.max)
                    stt(v, dil[:, :, 1:3, :], dil[:, :, 1:3, :], vdil[:, :, 2:4, :], A.bypass, A.max)
                    if exch:
                        stt(v, dil[:, :, 0:1, :], vdil[:, :, 0:1, :], vdil[:, :, 2:3, :], A.bypass, A.max)
                        stt(v, dil[:, :, 0:1, :], dil[:, :, 0:1, :], vdil[:, :, 1:2, :], A.bypass, A.max)
                        stt(v, dil[:, :, 3:4, :], vdil[:, :, 3:4, :], vdil[:, :, 5:6, :], A.bypass, A.max)
                        stt(v, dil[:, :, 3:4, :], dil[:, :, 3:4, :], vdil[:, :, 4:5, :], A.bypass, A.max)
                    else:
                        stt(v, dil[:, :, 0:1, :], vdil[:, :, 1:2, :], vdil[:, :, 2:3, :], A.bypass, A.max)
                        stt(v, dil[:, :, 3:4, :], vdil[:, :, 3:4, :], vdil[:, :, 4:5, :], A.bypass, A.max)
                    stt(v, dil[:, :, :, :], dil[:, :, :, :], wk[:, :, :, :], A.bypass, A.mult)
                    stt(v, st[:, :, :, :], st[:, :, :, :], dil[:, :, :, :], A.bypass, A.max)
                o32 = phy.tile([128, NB, 4, W], F32, tag="o32", name="o32")
                v.tensor_tensor(out=o32[:, :, :, :], in0=st[:, :, :, 1 : 1 + W],
                                in1=st[:, :, :, 1 : 1 + W], op=A.max)
                nc.sync.dma_start(
                    out=out[:, :, :].rearrange("b (p r) x -> p b r x", r=4),
                    in_=o32[:, :, :, :],
                )
    nc.finalize()
    return nc


# ---------------------------------------------------------------------------
# walrus 1-sync-wait-per-instruction workaround (BIR JSON post-pass)
# ---------------------------------------------------------------------------
import json as _json

_ws_counter = [0]


def _split_instruction_list(instrs):
    out = []
    for ins in instrs:
        si = ins.get("sync_info")
        waits = (si or {}).get("on_wait") or []
        if len(waits) > 1:
            for wcond in waits[:-1]:
                _ws_counter[0] += 1
                out.append({
                    "debug": ins.get("debug", 0),
                    "engine": ins["engine"],
                    "ins": [],
                    "name": f"I-waitsplit-{_ws_counter[0]}",
                    "opcode": "NoOp",
                    "outs": [],
                    "sync_info": {"on_wait": [wcond], "on_update": []},
                })
            si = dict(si)
            si["on_wait"] = [waits[-1]]
            ins = dict(ins)
            ins["sync_info"] = si
        out.append(ins)
    return out


def _walk_split(obj):
    if isinstance(obj, dict):
        for k, val in obj.items():
            if k == "instructions" and isinstance(val, list):
                obj[k] = _split_instruction_list(val)
            else:
                _walk_split(val)
    elif isinstance(obj, list):
        for val in obj:
            _walk_split(val)


def _split_multiwait_bir(bir_json):
    j = _json.loads(bir_json)
    _walk_split(j)
    return _json.dumps(j).encode()


_patched = [False]


def _install_bir_patch():
    if _patched[0]:
        return
    _patched[0] = True
    import concourse.bass_utils as bu

    orig = bu.compile_bir_kernel

    def patched(bir_json, tmpdir, neff_name="file.neff"):
        return orig(_split_multiwait_bir(bir_json), tmpdir, neff_name)

    bu.compile_bir_kernel = patched
    try:
        import concourse.bass2jax as b2j

        b2j.compile_bir_kernel = patched
    except Exception:
        pass


# ---------------------------------------------------------------------------
# host entry point
# ---------------------------------------------------------------------------
_cache = {}


def _get_program(rep=1):
    key = ("nc", rep)
    if key not in _cache:
        _install_bir_patch()
        _cache[key] = build(rep=rep)
    return _cache[key]


def make_in_maps(x):
    x = np.asarray(x, dtype=np.float32)
    xpad = np.pad(x, ((0, 0), (0, 0), (PAD, PAD), (PAD, PAD)), mode="reflect")
    return [
        {"xp": np.ascontiguousarray(xpad[i * NB : (i + 1) * NB])}
        for i in range(NCORES)
    ]


def kernel(x):
    """x: [16,3,512,512] float32 -> edges [16,1,512,512] float32."""
    from concourse.bass_utils import run_bass_kernel_spmd

    x = np.asarray(x, dtype=np.float32)
    B = x.shape[0]
    assert x.shape == (NCORES * NB, 3, H, W), x.shape
    nc = _get_program()
    in_maps = make_in_maps(x)
    res = run_bass_kernel_spmd(nc, in_maps, core_ids=list(range(NCORES)))
    out = np.empty((B, 1, H, W), np.float32)
    for i in range(NCORES):
        out[i * NB : (i + 1) * NB, 0] = res.results[i]["edges"]
    return out
